# revision 20
# baseline (speedup 1.0000x reference)
"""Multi-head self-attention (B=2, S=2048, E=1024, H=16) on 8 Trainium2 cores.

Sharding: 2D (batch x head-group). Core c handles batch b = c // 4 and head
group g = c % 4 (4 heads, 256 embed columns). Each core computes its QKV
projection slices, fused attention for its 4 heads, and a partial output
projection (attn_g @ Wo[g_slice]); the host sums the 4 partials per batch
(the head-concat contraction) and stacks the 2 batches.

Device layout choices (all matmul contractions land on the partition axis,
so no on-device transposes are needed anywhere):
  - host supplies x^T per batch for q/k/v, bf16, pre-tiled into the exact
    [128, KC, span] blocks the kernel loads (every DMA fully contiguous)
  - Q/K projections produce Q^T/K^T  [d', S] (head-dim on partitions)
  - V projection produces V [S, d'] (seq on partitions), stored interleaved
    with a ones column per head ([V_h | 1] * 4) so that P @ [V_h | 1] yields
    both the attention numerator and the softmax denominator in one pass
  - logits^T tiles [j, i] feed exp (ScalarE, no max-subtraction: |logits|<~8)
    giving P^T tiles which are exactly the rhs layout P@V needs
  - 1/8 scaling and biases are folded in on the host / into copy-backs;
    bv is folded via P @ [V + 1 bv^T] = P@V + bv (softmax rows sum to 1)

Schedule (v2): the prologue is cut to the minimum needed for attention
jc0 of unit (0,0): K span (0,0), V chunk 0, Q tile (0,0,0), with the DMAs
emitted in exact dependency order.  Every other projection span rides as
a just-in-time filler under the exp stream, paced so span s of K lands
before jc=4s and V chunk sg before jc=2sg.  The output projection is
split per 128-contraction half: the cc=0 half runs early (as soon as
OT[:,0,...] exists) into SBUF partials with the bias folded in; the cc=1
half + combine + store trail each OT[:,1,...] write, so the post-loop
tail is only the last 8 half-MMs instead of 16 full out-projections.
"""

import math
import numpy as np
import ml_dtypes

BF16 = ml_dtypes.bfloat16

P = 128
S = 2048
E = 1024
GE = 256          # embed columns per core (4 heads x 64)
KC = 8            # contraction chunks of 128 over E
JC = 16           # key chunks of 128 over S
IT = 4            # query tiles of 512 over S
NCORES = 8

_NC = None        # cached compiled program


def _build_program():
    import concourse.tile as tile
    from concourse import bacc, mybir

    F32 = mybir.dt.float32
    BF = mybir.dt.bfloat16
    Exp = mybir.ActivationFunctionType.Exp
    mult = mybir.AluOpType.mult
    add = mybir.AluOpType.add

    nc = bacc.Bacc(
        "TRN2",
        target_bir_lowering=False,
        debug=False,
        enable_asserts=False,
        num_devices=NCORES,
    )

    # x inputs come pre-tiled from the host so every SBUF load is one
    # fully contiguous DMA: [g, p, kc, s] = x^T[kc*128+p, g*W+s]
    d_xq = nc.dram_tensor("xqT", [4, P, KC, 512], BF, kind="ExternalInput")
    d_xk = nc.dram_tensor("xkT", [4, P, KC, 512], BF, kind="ExternalInput")
    d_xv = nc.dram_tensor("xvT", [8, P, KC, GE], BF, kind="ExternalInput")
    d_wq = nc.dram_tensor("wq", [P, KC, GE], BF, kind="ExternalInput")
    d_wk = nc.dram_tensor("wk", [P, KC, GE], BF, kind="ExternalInput")
    d_wv = nc.dram_tensor("wv", [P, KC, GE], BF, kind="ExternalInput")
    d_wo = nc.dram_tensor("wo", [P, 2, E], BF, kind="ExternalInput")
    d_bq = nc.dram_tensor("bqs", [P, 2], F32, kind="ExternalInput")
    d_bk = nc.dram_tensor("bks", [P, 2], F32, kind="ExternalInput")
    d_bv = nc.dram_tensor("bvb", [P, GE], F32, kind="ExternalInput")
    d_bo = nc.dram_tensor("bob", [P, E], F32, kind="ExternalInput")
    d_y = nc.dram_tensor("y", [S, E], F32, kind="ExternalOutput")

    with tile.TileContext(nc) as tc:
        with (
            tc.tile_pool(name="w", bufs=1) as wpool,
            tc.tile_pool(name="x", bufs=1) as xpool,
            tc.tile_pool(name="persist", bufs=1) as pers,
            tc.tile_pool(name="pt", bufs=8) as ptp,
            tc.tile_pool(name="sm", bufs=3) as sm,
            tc.tile_pool(name="y", bufs=4) as yp,
            tc.tile_pool(name="y0", bufs=16) as y0p,
            tc.tile_pool(name="psA", bufs=2, space="PSUM") as psA,
            tc.tile_pool(name="psB", bufs=4, space="PSUM") as psB,
        ):
            # ---- weights / biases resident in SBUF (DMAs sequenced below) --
            wq_t = wpool.tile([P, KC, GE], BF, tag="wq")
            wk_t = wpool.tile([P, KC, GE], BF, tag="wk")
            wv_t = wpool.tile([P, KC, GE], BF, tag="wv")
            wo_t = wpool.tile([P, 2, E], BF, tag="wo")
            bq_t = wpool.tile([P, 2], F32, tag="bq")
            bk_t = wpool.tile([P, 2], F32, tag="bk")
            bv_t = wpool.tile([P, GE], F32, tag="bv")
            bo_t = wpool.tile([P, E], F32, tag="bo")

            # ---- persistent activations ----
            QT = pers.tile([P, 2, S], BF, tag="QT")   # [d'(2x128), S]
            KT = pers.tile([P, 2, S], BF, tag="KT")
            V1 = pers.tile([P, JC, 260], BF, tag="V1")  # [S(16x128), (V_h|1)*4]
            OT = pers.tile([P, 2, S], BF, tag="OT")

            # ones columns (col 64 of each 65-wide head block)
            nc.vector.memset(V1[:, :, 64::65], 1.0)
            ones_t = wpool.tile([1, 64], F32, tag="ones")
            nc.vector.memset(ones_t[:], 1.0)

            def qk_pair(w_t, b_t, dst, xd, tt, th):
                # one [128, 512] span of a Q/K projection for BOTH head-pair
                # slabs (c=0,1) off a single 1 MB contiguous x load: 2x4
                # steps of 2 accumulating MMs; step 3 of each slab adds the
                # bias on copy-back.  Sharing the load halves the x traffic.
                s0 = (tt * 2 + th) * 512
                st = {}

                def prefetch():
                    st["xs"] = xpool.tile([P, KC, 512], BF, tag="xs", bufs=4,
                                          name=f"xs_{tt}_{th}")
                    # two half loads: subtile deps let steps 0-1 (kc 0-3)
                    # start as soon as the first 0.5 MB lands
                    nc.sync.dma_start(st["xs"][:, 0:4, :],
                                      xd[tt * 2 + th, :, 0:4, :])
                    nc.sync.dma_start(st["xs"][:, 4:8, :],
                                      xd[tt * 2 + th, :, 4:8, :])

                def step(c, i):
                    if i == 0:
                        st[c] = psB.tile([P, 512], F32, tag="acc",
                                         name=f"qkps_{c}_{tt}_{th}")
                    for kc in (2 * i, 2 * i + 1):
                        nc.tensor.matmul(
                            st[c][:],
                            lhsT=w_t[:, kc, c * P:(c + 1) * P],
                            rhs=st["xs"][:, kc, :],
                            start=(kc == 0), stop=(kc == KC - 1),
                        )
                    if i == 3:
                        nc.vector.tensor_scalar_add(
                            dst[:, c, s0:s0 + 512], st[c][:],
                            b_t[:, c:c + 1])

                return (prefetch,
                        [lambda i=i: step(0, i) for i in range(4)],
                        [lambda i=i: step(1, i) for i in range(4)])

            def v_group(sg):
                # V projection for two s-chunks -> V1 (interleaved V|1 cols):
                # a 0.5 MB just-in-time slice prefetch plus 2 steps of 8 MMs,
                # each step copying back its own s-chunk (so PV(jc) may run as
                # soon as the slot-jc step has finished).
                st = {}

                def prefetch():
                    st["vs"] = xpool.tile([P, KC, GE], BF, tag="vs", bufs=4,
                                          name=f"vs_{sg}")
                    nc.sync.dma_start(st["vs"][:], d_xv[sg])

                def step(i2):
                    if i2 == 0:
                        st["ps"] = psB.tile([P, 512], F32, tag="acc",
                                            name=f"vps_{sg}")
                    sc = sg * 2 + i2
                    for kc in range(KC):
                        nc.tensor.matmul(
                            st["ps"][:, i2 * GE:(i2 + 1) * GE],
                            lhsT=st["vs"][:, kc, i2 * P:(i2 + 1) * P],
                            rhs=wv_t[:, kc, :],
                            start=(kc == 0), stop=(kc == KC - 1),
                        )
                    for h in range(4):
                        nc.vector.tensor_tensor(
                            V1[:, sc, 65 * h:65 * h + 64],
                            st["ps"][:, i2 * GE + 64 * h:
                                     i2 * GE + 64 * (h + 1)],
                            bv_t[:, 64 * h:64 * (h + 1)],
                            add,
                        )

                return prefetch, [lambda i=i: step(i) for i in range(2)]

            # ---- output projection, split per 128-row contraction half ----
            # cc=0 half: y0 = OT[:,0]^T @ Wo[0] + bo  (SBUF partial, can run
            # as soon as unit (0, t) wrote its OT slab).  cc=1 half:
            # y = y0 + OT[:,1]^T @ Wo[1], then store.
            y0sbs = {}
            ysbs = {}

            def out0_step(sc, nt):
                def f():
                    if sc not in y0sbs:
                        # bf16 partial: halves SBUF footprint; adds ~1e-3
                        # relative error on half the contraction only
                        y0sbs[sc] = y0p.tile([P, E], BF, tag="y0",
                                             name=f"y0sb_{sc}")
                    ps = psB.tile([P, 512], F32, tag="acc",
                                  name=f"o0ps_{sc}_{nt}")
                    nc.tensor.matmul(
                        ps[:],
                        lhsT=OT[:, 0, sc * P:(sc + 1) * P],
                        rhs=wo_t[:, 0, nt * 512:(nt + 1) * 512],
                        start=True, stop=True,
                    )
                    nc.vector.tensor_tensor(
                        y0sbs[sc][:, nt * 512:(nt + 1) * 512], ps[:],
                        bo_t[:, nt * 512:(nt + 1) * 512], add)
                return f

            def out1_step(sc, nt):
                def f():
                    if sc not in ysbs:
                        ysbs[sc] = yp.tile([P, E], F32, tag="ysb",
                                           name=f"ysb_{sc}")
                    ps = psB.tile([P, 512], F32, tag="acc",
                                  name=f"o1ps_{sc}_{nt}")
                    nc.tensor.matmul(
                        ps[:],
                        lhsT=OT[:, 1, sc * P:(sc + 1) * P],
                        rhs=wo_t[:, 1, nt * 512:(nt + 1) * 512],
                        start=True, stop=True,
                    )
                    nc.vector.tensor_tensor(
                        ysbs[sc][:, nt * 512:(nt + 1) * 512], ps[:],
                        y0sbs[sc][:, nt * 512:(nt + 1) * 512], add)
                    if nt == 1:
                        y0sbs.pop(sc)
                        nc.sync.dma_start(
                            d_y[sc * P:(sc + 1) * P, :], ysbs.pop(sc)[:])
                return f

            def out0_fill(t):
                return [out0_step(sc, nt)
                        for sc in range(4 * t, 4 * t + 4) for nt in range(2)]

            def out1_fill(t):
                return [out1_step(sc, nt)
                        for sc in range(4 * t, 4 * t + 4) for nt in range(2)]

            # ---- attention unit: 16 jc iterations of logits -> exp -> PV,
            # with filler steps (projections / out-proj halves) paced so the
            # PE's slack under the exp stream is filled without starving it.
            def attn_unit(c, t, fillers, lead=0, fast=False, slots=None):
                # lead: filler-free jc slots at unit start, so this unit's
                # filler TTs don't queue on the DVE behind the previous
                # unit's epilogue chain (strict FIFO) and hold psB slots.
                # slots: explicit per-jc filler lists (16 lists) for units
                # with hard in-unit deadlines; overrides ceil pacing.
                fillers = list(fillers)
                tsl = slice(t * 512, (t + 1) * 512)
                pO0 = psB.tile([65, 512], F32, tag="acc")
                pO1 = psB.tile([65, 512], F32, tag="acc")
                for jc in range(JC):
                    if slots is not None:
                        for f in slots[jc]:
                            f()
                    else:
                        if jc < lead:
                            npop = 0
                        else:
                            npop = -(-len(fillers) // (JC - jc))  # ceil
                        for _ in range(min(npop, len(fillers))):
                            fillers.pop(0)()
                    jsl = slice(jc * P, (jc + 1) * P)
                    pL = psA.tile([P, 1024], F32, tag="big")
                    nc.tensor.matmul(
                        pL[:, 0:512],
                        lhsT=KT[0:64, c, jsl], rhs=QT[0:64, c, tsl],
                        start=True, stop=True,
                    )
                    nc.tensor.matmul(
                        pL[:, 512:1024],
                        lhsT=KT[64:128, c, jsl], rhs=QT[64:128, c, tsl],
                        start=True, stop=True,
                    )
                    pt = ptp.tile([P, 1024], BF, tag="pt")
                    nc.scalar.activation(pt[:], pL[:], Exp)
                    nc.tensor.matmul(
                        pO0[:], lhsT=V1[:, jc, 130 * c:130 * c + 65],
                        rhs=pt[:, 0:512],
                        start=(jc == 0), stop=(jc == JC - 1),
                    )
                    nc.tensor.matmul(
                        pO1[:], lhsT=V1[:, jc, 130 * c + 65:130 * c + 130],
                        rhs=pt[:, 512:1024],
                        start=(jc == 0), stop=(jc == JC - 1),
                    )
                # normalize: OT_h = pO[0:64] / pO[64]  (row 64 = sum of P).
                # Copy the accumulator to SBUF (frees the bank, and gives the
                # multiply an SBUF operand), take the exact reciprocal of the
                # sum row in place on the DVE, and broadcast it across 64
                # partitions with a gpsimd partition broadcast.
                if fast:
                    # latency-optimized variant for the LAST unit, where the
                    # chain is fully exposed: both heads' copies issue first
                    # (pipelined on the DVE), and 1/sum is broadcast with a
                    # K=1 PE outer product (the PE is idle here) instead of
                    # the ~1us gpsimd broadcast.  A TT may read only ONE
                    # psum operand, so osb (SBUF) x rbp (PSUM) -> OT.
                    osbs = []
                    for hp, pO in ((0, pO0), (1, pO1)):
                        osb = sm.tile([65, 512], F32, tag="osb", bufs=3,
                                      name=f"osbf_{hp}")
                        nc.vector.tensor_copy(osb[:], pO[:])
                        osbs.append(osb)
                    recs = []
                    for hp in (0, 1):
                        r0 = sm.tile([1, 512], F32, tag="rec0", bufs=3,
                                     name=f"rec0f_{hp}")
                        nc.sync.dma_start(r0[:], osbs[hp][64:65, :])
                        recs.append(r0)
                    # warm-keeper: one junk MM gated on the rec0 DMA keeps
                    # the PE from re-throttling during the chain, so the
                    # broadcast and out-projection MMs run at 2.4 GHz
                    nc.tensor.matmul(psB.tile([P, 512], F32, tag="acc",
                                              name="warmtail")[:],
                                     lhsT=recs[0][0:1, 0:128],
                                     rhs=recs[0][:], start=True, stop=True)
                    rins = []
                    for hp in (0, 1):
                        ri = sm.tile([1, 512], F32, tag="rin", bufs=3,
                                     name=f"rinf_{hp}")
                        nc.vector.reciprocal_approx_fast(ri[:], recs[hp][:])
                        rins.append(ri)
                    rbps = []
                    for hp in (0, 1):
                        rb = psA.tile([64, 512], F32, tag="big",
                                      name=f"rbpf_{hp}")
                        nc.tensor.matmul(rb[:], lhsT=ones_t[0:1, :],
                                         rhs=rins[hp][:],
                                         start=True, stop=True)
                        rbps.append(rb)
                    nc.vector.tensor_tensor(OT[0:64, c, tsl],
                                            osbs[0][0:64, :], rbps[0][:],
                                            mult)
                    ott = sm.tile([64, 512], BF, tag="ott", bufs=3)
                    nc.vector.tensor_tensor(ott[:], osbs[1][0:64, :],
                                            rbps[1][:], mult)
                    nc.sync.dma_start(OT[64:128, c, tsl], ott[:])
                    while fillers:
                        fillers.pop(0)()
                    return
                # stage-pipelined across the two heads: both psum->sbuf
                # copies issue back-to-back first, releasing the pO psB
                # slots before the next unit's accumulators need them (the
                # old per-head chain parked copy1 behind head0's gpsimd
                # wait in the DVE FIFO, stalling the next unit ~2us)
                osbs = []
                for hp, pO in ((0, pO0), (1, pO1)):
                    osb = sm.tile([65, 512], F32, tag="osb", bufs=3,
                                  name=f"osb_{hp}")
                    nc.vector.tensor_copy(osb[:], pO[:])
                    osbs.append(osb)
                recs = []
                for hp in (0, 1):
                    rec0 = sm.tile([1, 512], F32, tag="rec0", bufs=3,
                                   name=f"rec0_{hp}")
                    nc.sync.dma_start(rec0[:], osbs[hp][64:65, :])
                    recs.append(rec0)
                rins = []
                for hp in (0, 1):
                    rin = sm.tile([1, 512], F32, tag="rin", bufs=3,
                                  name=f"rin_{hp}")
                    nc.vector.reciprocal_approx_fast(rin[:], recs[hp][:])
                    rins.append(rin)
                rbss = []
                for hp in (0, 1):
                    rbs = sm.tile([64, 512], F32, tag="rbs", bufs=3,
                                  name=f"rbs_{hp}")
                    nc.gpsimd.partition_broadcast(rbs[:], rins[hp][:])
                    rbss.append(rbs)
                # head 0's partitions line up: write OT rows 0-63 directly
                nc.vector.tensor_tensor(OT[0:64, c, tsl],
                                        osbs[0][0:64, :], rbss[0][:], mult)
                # head 1 rows 64-127 need a cross-partition move (DMA)
                ott = sm.tile([64, 512], BF, tag="ott", bufs=3)
                nc.vector.tensor_tensor(ott[:], osbs[1][0:64, :],
                                        rbss[1][:], mult)
                nc.sync.dma_start(OT[64:128, c, tsl], ott[:])
                while fillers:
                    fillers.pop(0)()

            # ---- prologue: exactly what attention jc0 of unit (0,0)
            # needs, DMAs emitted in dependency order.  Only the c=0 halves
            # of the (0,0) span pairs run here (first-logits critical path);
            # the c=1 halves ride as early U00 fillers ----
            # DVE-paced warm-up: the HAM clock gate is cold (1.2 GHz) at
            # start and re-throttles after ~3.4us idle.  A ping-pong DVE
            # copy chain paces junk MMs across the initial DMA wait so the
            # PE is warm when the first real MMs fire.
            sa = xpool.tile([P, 512], BF, tag="scra", name="scr_a")
            sb = xpool.tile([P, 512], BF, tag="scrb", name="scr_b")
            nc.vector.memset(sa[:], 0.0)
            wps = psB.tile([P, 512], F32, tag="acc", name="warmps")
            for i in range(14):
                s_src, s_dst = (sa, sb) if i % 2 == 0 else (sb, sa)
                nc.vector.tensor_copy(s_dst[:], s_src[:])
                nc.tensor.matmul(wps[:], lhsT=s_dst[:, 0:128], rhs=s_dst[:],
                                 start=True, stop=True)

            # critical loads, Q first (its compute chain ends at the first
            # logits); biases deferred past the x data they don't gate
            nc.sync.dma_start(wq_t[:], d_wq[:])
            q00 = qk_pair(wq_t, bq_t, QT, d_xq, 0, 0)
            q00[0]()
            nc.sync.dma_start(wk_t[:], d_wk[:])
            k00 = qk_pair(wk_t, bk_t, KT, d_xk, 0, 0)
            k00[0]()
            nc.sync.dma_start(bq_t[:], d_bq[:])
            nc.sync.dma_start(bk_t[:], d_bk[:])
            nc.sync.dma_start(wv_t[:], d_wv[:])
            nc.sync.dma_start(bv_t[:], d_bv[:])
            v0_pre, v0 = v_group(0)
            v0_pre()
            # prefetches for U00's first consumers go out now, behind the
            # critical loads in the DMA FIFO
            vg = [None] + [v_group(sg) for sg in range(1, 8)]
            k01 = qk_pair(wk_t, bk_t, KT, d_xk, 0, 1)
            vg[1][0]()
            k01[0]()
            for s in q00[1] + k00[1] + v0:
                s()

            # ---- unit (0,0): carries the c=1 halves of the (0,0) pairs
            # (cheap, their x is resident), V chunks 1-7 (deadline jc=2sg),
            # and the c=0 K spans (deadline jc=4s) + Q span (0,1) c=0
            # (deadline: unit (0,1) start).  The far-deadline c=1 K/Q spans
            # are NOT here: they run in the ACT-paced later units, off
            # re-loaded x (those units have both PE and DMA slack) ----
            k10 = qk_pair(wk_t, bk_t, KT, d_xk, 1, 0)
            k11 = qk_pair(wk_t, bk_t, KT, d_xk, 1, 1)
            q01 = qk_pair(wq_t, bq_t, QT, d_xq, 0, 1)
            vg[2][0](); k10[0](); vg[3][0](); vg[4][0]()
            k11[0](); vg[5][0](); q01[0](); vg[6][0](); vg[7][0]()
            nc.sync.dma_start(wo_t[:], d_wo[:])
            nc.sync.dma_start(bo_t[:], d_bo[:])
            # explicit slots: V chunk sg must land by jc=2sg, K span s
            # (c=0, incl. the step-3 copyback) by jc=4s; the resident-x c=1
            # halves and Q (0,1) c=0 fill the remaining capacity
            V = [vg[s][1] if s else None for s in range(8)]
            slots00 = [
                [k00[2][0], k00[2][1]],
                [k00[2][2], k00[2][3], V[1][0]],
                [V[1][1], k01[1][0], k01[1][1]],
                [k01[1][2], k01[1][3], V[2][0]],
                [V[2][1], k10[1][0]],
                [k10[1][1], k10[1][2], V[3][0]],
                [V[3][1], k10[1][3]],
                [V[4][0], V[4][1]],
                [k11[1][0], k11[1][1]],
                [V[5][0], V[5][1]],
                [k11[1][2], k11[1][3]],
                [V[6][0], V[6][1]],
                [q00[2][0], q00[2][1]],
                [V[7][0], V[7][1], q00[2][2]],
                [q00[2][3], q01[1][0], q01[1][1]],
                [q01[1][2], q01[1][3]],
            ]
            attn_unit(0, 0, [], slots=slots00)

            # ---- unit (0,1): Q span (1,0), both slabs ----
            q10 = qk_pair(wq_t, bq_t, QT, d_xq, 1, 0)
            q10[0]()
            attn_unit(0, 1, q10[1] + q10[2], lead=3)

            # ---- remaining units: out-projection halves (<=12-16 fillers
            # per unit keeps the DVE off the per-jc critical path) plus the
            # c=1 K/Q spans off re-loaded x, placed just before their
            # consumers ----
            q11 = qk_pair(wq_t, bq_t, QT, d_xq, 1, 1)
            q11[0]()
            attn_unit(0, 2, q11[1] + q11[2] + out0_fill(0), lead=3)

            k01r = qk_pair(wk_t, bk_t, KT, d_xk, 0, 1)
            k01r[0]()
            attn_unit(0, 3, k01r[2] + q01[2] + out0_fill(1), lead=3)

            k10r = qk_pair(wk_t, bk_t, KT, d_xk, 1, 0)
            k11r = qk_pair(wk_t, bk_t, KT, d_xk, 1, 1)
            k10r[0](); k11r[0]()
            attn_unit(1, 0, k10r[2] + k11r[2] + out0_fill(2), lead=3)

            o10 = out1_fill(0)
            attn_unit(1, 1, out0_fill(3) + o10[:4], lead=3)
            attn_unit(1, 2, o10[4:] + out1_fill(1), lead=3)
            attn_unit(1, 3, out1_fill(2), lead=3, fast=True)
            for f in out1_fill(3):
                f()

    nc.compile()
    return nc


def _get_program():
    global _NC
    if _NC is None:
        _NC = _build_program()
    return _NC


def kernel(q, k, v, Wq, bq, Wk, bk, Wv, bv, Wo, bo):
    from concourse.bass_utils import run_bass_kernel_spmd

    q = np.asarray(q, np.float32)
    k = np.asarray(k, np.float32)
    v = np.asarray(v, np.float32)
    Wq = np.asarray(Wq, np.float32)
    Wk = np.asarray(Wk, np.float32)
    Wv = np.asarray(Wv, np.float32)
    Wo = np.asarray(Wo, np.float32)
    bq = np.asarray(bq, np.float32)
    bk = np.asarray(bk, np.float32)
    bv = np.asarray(bv, np.float32)
    bo = np.asarray(bo, np.float32)

    nc = _get_program()

    def tile_qk(xb):
        # [S, E] -> x^T tiled [4, 128, KC, 512]
        return np.ascontiguousarray(
            xb.T.reshape(KC, P, 4, 512).transpose(2, 1, 0, 3)).astype(BF16)

    def tile_v(xb):
        # [S, E] -> x^T tiled [8, 128, KC, 256]
        return np.ascontiguousarray(
            xb.T.reshape(KC, P, 8, GE).transpose(2, 1, 0, 3)).astype(BF16)

    xT = {"xqT": [tile_qk(q[b]) for b in range(2)],
          "xkT": [tile_qk(k[b]) for b in range(2)],
          "xvT": [tile_v(v[b]) for b in range(2)]}

    def wprep(W, scale=1.0):
        # [E, GE] slice -> [P, KC, GE] partition-major
        return [
            np.ascontiguousarray(
                (W[:, g * GE:(g + 1) * GE] * scale)
                .reshape(KC, P, GE).transpose(1, 0, 2)
            ).astype(BF16)
            for g in range(4)
        ]

    wq_g = wprep(Wq, 0.125)
    wk_g = wprep(Wk)
    wv_g = wprep(Wv)
    wo_g = [
        np.ascontiguousarray(
            Wo[g * GE:(g + 1) * GE, :].reshape(2, P, E).transpose(1, 0, 2)
        ).astype(BF16)
        for g in range(4)
    ]
    bq_g = [np.ascontiguousarray((bq[g * GE:(g + 1) * GE] * 0.125)
                                 .reshape(2, P).T).astype(np.float32)
            for g in range(4)]
    bk_g = [np.ascontiguousarray(bk[g * GE:(g + 1) * GE].reshape(2, P).T)
            .astype(np.float32) for g in range(4)]
    bv_g = [np.ascontiguousarray(np.broadcast_to(
        bv[g * GE:(g + 1) * GE].astype(np.float32), (P, GE))) for g in range(4)]
    bo_full = np.ascontiguousarray(
        np.broadcast_to(bo.astype(np.float32), (P, E)))
    bo_zero = np.zeros((P, E), np.float32)

    in_maps = []
    for c in range(NCORES):
        b, g = divmod(c, 4)
        in_maps.append({
            "xqT": xT["xqT"][b],
            "xkT": xT["xkT"][b],
            "xvT": xT["xvT"][b],
            "wq": wq_g[g], "wk": wk_g[g], "wv": wv_g[g], "wo": wo_g[g],
            "bqs": bq_g[g], "bks": bk_g[g], "bvb": bv_g[g],
            "bob": bo_full if g == 0 else bo_zero,
        })

    res = run_bass_kernel_spmd(nc, in_maps, list(range(NCORES)),
                               **_RUN_KWARGS)
    globals()["LAST_RESULTS"] = res

    parts = [res.results[c]["y"] for c in range(NCORES)]
    out = np.stack([
        parts[0] + parts[1] + parts[2] + parts[3],
        parts[4] + parts[5] + parts[6] + parts[7],
    ]).astype(np.float32)
    return out


# test-harness hooks (kernel.py itself never enables tracing)
_RUN_KWARGS = {}
LAST_RESULTS = None


# revision 22
# speedup vs baseline: 1.0340x; 1.0340x over previous
"""Multi-head self-attention (B=2, S=2048, E=1024, H=16) on 8 Trainium2 cores.

Sharding: 2D (batch x head-group). Core c handles batch b = c // 4 and head
group g = c % 4 (4 heads, 256 embed columns). Each core computes its QKV
projection slices, fused attention for its 4 heads, and a partial output
projection (attn_g @ Wo[g_slice]); the host sums the 4 partials per batch
(the head-concat contraction) and stacks the 2 batches.

Device layout choices (all matmul contractions land on the partition axis,
so no on-device transposes are needed anywhere):
  - host supplies x^T per batch for q/k/v, bf16, pre-tiled into the exact
    [128, KC, span] blocks the kernel loads (every DMA fully contiguous)
  - Q/K projections produce Q^T/K^T  [d', S] (head-dim on partitions)
  - V projection produces V [S, d'] (seq on partitions), stored interleaved
    with a ones column per head ([V_h | 1] * 4) so that P @ [V_h | 1] yields
    both the attention numerator and the softmax denominator in one pass
  - logits^T tiles [j, i] feed exp (ScalarE, no max-subtraction: |logits|<~8)
    giving P^T tiles which are exactly the rhs layout P@V needs
  - 1/8 scaling and biases are folded in on the host / into copy-backs;
    bv is folded via P @ [V + 1 bv^T] = P@V + bv (softmax rows sum to 1)

Schedule (v2): the prologue is cut to the minimum needed for attention
jc0 of unit (0,0): K span (0,0), V chunk 0, Q tile (0,0,0), with the DMAs
emitted in exact dependency order.  Every other projection span rides as
a just-in-time filler under the exp stream, paced so span s of K lands
before jc=4s and V chunk sg before jc=2sg.  The output projection is
split per 128-contraction half: the cc=0 half runs early (as soon as
OT[:,0,...] exists) into SBUF partials with the bias folded in; the cc=1
half + combine + store trail each OT[:,1,...] write, so the post-loop
tail is only the last 8 half-MMs instead of 16 full out-projections.
"""

import math
import numpy as np
import ml_dtypes

BF16 = ml_dtypes.bfloat16

P = 128
S = 2048
E = 1024
GE = 256          # embed columns per core (4 heads x 64)
KC = 8            # contraction chunks of 128 over E
JC = 16           # key chunks of 128 over S
IT = 4            # query tiles of 512 over S
NCORES = 8

_NC = None        # cached compiled program


def _build_program():
    import concourse.tile as tile
    from concourse import bacc, mybir

    F32 = mybir.dt.float32
    BF = mybir.dt.bfloat16
    Exp = mybir.ActivationFunctionType.Exp
    mult = mybir.AluOpType.mult
    add = mybir.AluOpType.add

    nc = bacc.Bacc(
        "TRN2",
        target_bir_lowering=False,
        debug=False,
        enable_asserts=False,
        num_devices=NCORES,
    )

    # x inputs come pre-tiled from the host so every SBUF load is one
    # fully contiguous DMA: [g, p, kc, s] = x^T[kc*128+p, g*W+s]
    d_xq = nc.dram_tensor("xqT", [4, P, KC, 512], BF, kind="ExternalInput")
    d_xk = nc.dram_tensor("xkT", [4, P, KC, 512], BF, kind="ExternalInput")
    d_xv = nc.dram_tensor("xvT", [8, P, KC, GE], BF, kind="ExternalInput")
    d_wq = nc.dram_tensor("wq", [P, KC, GE], BF, kind="ExternalInput")
    d_wk = nc.dram_tensor("wk", [P, KC, GE], BF, kind="ExternalInput")
    d_wv = nc.dram_tensor("wv", [P, KC, GE], BF, kind="ExternalInput")
    d_wo = nc.dram_tensor("wo", [P, 2, E], BF, kind="ExternalInput")
    d_bq = nc.dram_tensor("bqs", [P, 2], F32, kind="ExternalInput")
    d_bk = nc.dram_tensor("bks", [P, 2], F32, kind="ExternalInput")
    d_bv = nc.dram_tensor("bvb", [P, GE], F32, kind="ExternalInput")
    d_bo = nc.dram_tensor("bob", [P, E], F32, kind="ExternalInput")
    d_y = nc.dram_tensor("y", [S, E], F32, kind="ExternalOutput")

    with tile.TileContext(nc) as tc:
        with (
            tc.tile_pool(name="w", bufs=1) as wpool,
            tc.tile_pool(name="x", bufs=1) as xpool,
            tc.tile_pool(name="persist", bufs=1) as pers,
            tc.tile_pool(name="pt", bufs=8) as ptp,
            tc.tile_pool(name="sm", bufs=3) as sm,
            tc.tile_pool(name="y", bufs=4) as yp,
            tc.tile_pool(name="y0", bufs=16) as y0p,
            tc.tile_pool(name="psA", bufs=2, space="PSUM") as psA,
            tc.tile_pool(name="psB", bufs=4, space="PSUM") as psB,
        ):
            # ---- weights / biases resident in SBUF (DMAs sequenced below) --
            wq_t = wpool.tile([P, KC, GE], BF, tag="wq")
            wk_t = wpool.tile([P, KC, GE], BF, tag="wk")
            wv_t = wpool.tile([P, KC, GE], BF, tag="wv")
            wo_t = wpool.tile([P, 2, E], BF, tag="wo")
            bq_t = wpool.tile([P, 2], F32, tag="bq")
            bk_t = wpool.tile([P, 2], F32, tag="bk")
            bv_t = wpool.tile([P, GE], F32, tag="bv")
            bo_t = wpool.tile([P, E], F32, tag="bo")

            # ---- persistent activations ----
            QT = pers.tile([P, 2, S], BF, tag="QT")   # [d'(2x128), S]
            KT = pers.tile([P, 2, S], BF, tag="KT")
            V1 = pers.tile([P, JC, 260], BF, tag="V1")  # [S(16x128), (V_h|1)*4]
            OT = pers.tile([P, 2, S], BF, tag="OT")

            # ones columns (col 64 of each 65-wide head block)
            nc.vector.memset(V1[:, :, 64::65], 1.0)
            ones_t = wpool.tile([1, 64], F32, tag="ones")
            nc.vector.memset(ones_t[:], 1.0)

            def qk_pair(w_t, b_t, dst, xd, tt, th):
                # one [128, 512] span of a Q/K projection for BOTH head-pair
                # slabs (c=0,1) off a single 1 MB contiguous x load: 2x4
                # steps of 2 accumulating MMs; step 3 of each slab adds the
                # bias on copy-back.  Sharing the load halves the x traffic.
                s0 = (tt * 2 + th) * 512
                st = {}

                def prefetch():
                    st["xs"] = xpool.tile([P, KC, 512], BF, tag="xs", bufs=4,
                                          name=f"xs_{tt}_{th}")
                    nc.sync.dma_start(st["xs"][:], xd[tt * 2 + th])

                def step(c, i):
                    if i == 0:
                        st[c] = psB.tile([P, 512], F32, tag="acc",
                                         name=f"qkps_{c}_{tt}_{th}")
                    for kc in (2 * i, 2 * i + 1):
                        nc.tensor.matmul(
                            st[c][:],
                            lhsT=w_t[:, kc, c * P:(c + 1) * P],
                            rhs=st["xs"][:, kc, :],
                            start=(kc == 0), stop=(kc == KC - 1),
                        )
                    if i == 3:
                        nc.vector.tensor_scalar_add(
                            dst[:, c, s0:s0 + 512], st[c][:],
                            b_t[:, c:c + 1])

                return (prefetch,
                        [lambda i=i: step(0, i) for i in range(4)],
                        [lambda i=i: step(1, i) for i in range(4)])

            def v_group(sg):
                # V projection for two s-chunks -> V1 (interleaved V|1 cols):
                # a 0.5 MB just-in-time slice prefetch plus 2 steps of 8 MMs,
                # each step copying back its own s-chunk (so PV(jc) may run as
                # soon as the slot-jc step has finished).
                st = {}

                def prefetch():
                    st["vs"] = xpool.tile([P, KC, GE], BF, tag="vs", bufs=4,
                                          name=f"vs_{sg}")
                    nc.sync.dma_start(st["vs"][:], d_xv[sg])

                def step(i2):
                    if i2 == 0:
                        st["ps"] = psB.tile([P, 512], F32, tag="acc",
                                            name=f"vps_{sg}")
                    sc = sg * 2 + i2
                    for kc in range(KC):
                        nc.tensor.matmul(
                            st["ps"][:, i2 * GE:(i2 + 1) * GE],
                            lhsT=st["vs"][:, kc, i2 * P:(i2 + 1) * P],
                            rhs=wv_t[:, kc, :],
                            start=(kc == 0), stop=(kc == KC - 1),
                        )
                    for h in range(4):
                        nc.vector.tensor_tensor(
                            V1[:, sc, 65 * h:65 * h + 64],
                            st["ps"][:, i2 * GE + 64 * h:
                                     i2 * GE + 64 * (h + 1)],
                            bv_t[:, 64 * h:64 * (h + 1)],
                            add,
                        )

                return prefetch, [lambda i=i: step(i) for i in range(2)]

            # ---- output projection, split per 128-row contraction half ----
            # cc=0 half: y0 = OT[:,0]^T @ Wo[0] + bo  (SBUF partial, can run
            # as soon as unit (0, t) wrote its OT slab).  cc=1 half:
            # y = y0 + OT[:,1]^T @ Wo[1], then store.
            y0sbs = {}
            ysbs = {}

            def out0_step(sc, nt):
                def f():
                    if sc not in y0sbs:
                        # bf16 partial: halves SBUF footprint; adds ~1e-3
                        # relative error on half the contraction only
                        y0sbs[sc] = y0p.tile([P, E], BF, tag="y0",
                                             name=f"y0sb_{sc}")
                    ps = psB.tile([P, 512], F32, tag="acc",
                                  name=f"o0ps_{sc}_{nt}")
                    nc.tensor.matmul(
                        ps[:],
                        lhsT=OT[:, 0, sc * P:(sc + 1) * P],
                        rhs=wo_t[:, 0, nt * 512:(nt + 1) * 512],
                        start=True, stop=True,
                    )
                    nc.vector.tensor_tensor(
                        y0sbs[sc][:, nt * 512:(nt + 1) * 512], ps[:],
                        bo_t[:, nt * 512:(nt + 1) * 512], add)
                return f

            def out1_step(sc, nt):
                def f():
                    if sc not in ysbs:
                        ysbs[sc] = yp.tile([P, E], F32, tag="ysb",
                                           name=f"ysb_{sc}")
                    ps = psB.tile([P, 512], F32, tag="acc",
                                  name=f"o1ps_{sc}_{nt}")
                    nc.tensor.matmul(
                        ps[:],
                        lhsT=OT[:, 1, sc * P:(sc + 1) * P],
                        rhs=wo_t[:, 1, nt * 512:(nt + 1) * 512],
                        start=True, stop=True,
                    )
                    nc.vector.tensor_tensor(
                        ysbs[sc][:, nt * 512:(nt + 1) * 512], ps[:],
                        y0sbs[sc][:, nt * 512:(nt + 1) * 512], add)
                    if nt == 1:
                        y0sbs.pop(sc)
                        nc.sync.dma_start(
                            d_y[sc * P:(sc + 1) * P, :], ysbs.pop(sc)[:])
                return f

            def out0_fill(t):
                return [out0_step(sc, nt)
                        for sc in range(4 * t, 4 * t + 4) for nt in range(2)]

            def out1_fill(t):
                return [out1_step(sc, nt)
                        for sc in range(4 * t, 4 * t + 4) for nt in range(2)]

            # ---- attention unit: 16 jc iterations of logits -> exp -> PV,
            # with filler steps (projections / out-proj halves) paced so the
            # PE's slack under the exp stream is filled without starving it.
            def attn_unit(c, t, fillers, lead=0, fast=False, slots=None):
                # lead: filler-free jc slots at unit start, so this unit's
                # filler TTs don't queue on the DVE behind the previous
                # unit's epilogue chain (strict FIFO) and hold psB slots.
                # slots: explicit per-jc filler lists (16 lists) for units
                # with hard in-unit deadlines; overrides ceil pacing.
                fillers = list(fillers)
                tsl = slice(t * 512, (t + 1) * 512)
                pO0 = psB.tile([65, 512], F32, tag="acc")
                pO1 = psB.tile([65, 512], F32, tag="acc")
                for jc in range(JC):
                    if slots is not None:
                        for f in slots[jc]:
                            f()
                    else:
                        if jc < lead:
                            npop = 0
                        else:
                            npop = -(-len(fillers) // (JC - jc))  # ceil
                        for _ in range(min(npop, len(fillers))):
                            fillers.pop(0)()
                    jsl = slice(jc * P, (jc + 1) * P)
                    pL = psA.tile([P, 1024], F32, tag="big")
                    nc.tensor.matmul(
                        pL[:, 0:512],
                        lhsT=KT[0:64, c, jsl], rhs=QT[0:64, c, tsl],
                        start=True, stop=True,
                    )
                    nc.tensor.matmul(
                        pL[:, 512:1024],
                        lhsT=KT[64:128, c, jsl], rhs=QT[64:128, c, tsl],
                        start=True, stop=True,
                    )
                    pt = ptp.tile([P, 1024], BF, tag="pt")
                    nc.scalar.activation(pt[:], pL[:], Exp)
                    nc.tensor.matmul(
                        pO0[:], lhsT=V1[:, jc, 130 * c:130 * c + 65],
                        rhs=pt[:, 0:512],
                        start=(jc == 0), stop=(jc == JC - 1),
                    )
                    nc.tensor.matmul(
                        pO1[:], lhsT=V1[:, jc, 130 * c + 65:130 * c + 130],
                        rhs=pt[:, 512:1024],
                        start=(jc == 0), stop=(jc == JC - 1),
                    )
                # normalize: OT_h = pO[0:64] / pO[64]  (row 64 = sum of P).
                # Copy the accumulator to SBUF (frees the bank, and gives the
                # multiply an SBUF operand), take the exact reciprocal of the
                # sum row in place on the DVE, and broadcast it across 64
                # partitions with a gpsimd partition broadcast.
                if fast:
                    # latency-optimized variant for the LAST unit, where the
                    # chain is fully exposed: both heads' copies issue first
                    # (pipelined on the DVE), and 1/sum is broadcast with a
                    # K=1 PE outer product (the PE is idle here) instead of
                    # the ~1us gpsimd broadcast.  A TT may read only ONE
                    # psum operand, so osb (SBUF) x rbp (PSUM) -> OT.
                    osbs = []
                    for hp, pO in ((0, pO0), (1, pO1)):
                        osb = sm.tile([65, 512], F32, tag="osb", bufs=3,
                                      name=f"osbf_{hp}")
                        nc.vector.tensor_copy(osb[:], pO[:])
                        osbs.append(osb)
                    recs = []
                    for hp in (0, 1):
                        r0 = sm.tile([1, 512], F32, tag="rec0", bufs=3,
                                     name=f"rec0f_{hp}")
                        nc.sync.dma_start(r0[:], osbs[hp][64:65, :])
                        recs.append(r0)
                    # warm-keepers: junk MMs gated on successive chain
                    # stages keep the PE clock from re-throttling during
                    # the exposed chain, so the trailing out-projection MMs
                    # run at 2.4 GHz instead of 1.2
                    def keeper(dep, nm):
                        nc.tensor.matmul(
                            psB.tile([P, 512], F32, tag="acc",
                                     name=nm)[0:64, :],
                            lhsT=dep[0:1, 0:64], rhs=dep[0:1, :],
                            start=True, stop=True)
                    keeper(osbs[0], "wt0")
                    rins = []
                    for hp in (0, 1):
                        ri = sm.tile([1, 512], F32, tag="rin", bufs=3,
                                     name=f"rinf_{hp}")
                        nc.vector.reciprocal_approx_fast(ri[:], recs[hp][:])
                        rins.append(ri)
                    keeper(rins[0], "wt1")
                    rbss = []
                    for hp in (0, 1):
                        rbs = sm.tile([64, 512], F32, tag="rbs", bufs=3,
                                      name=f"rbsf_{hp}")
                        nc.gpsimd.partition_broadcast(rbs[:], rins[hp][:])
                        rbss.append(rbs)
                    keeper(rbss[0], "wt2")
                    nc.vector.tensor_tensor(OT[0:64, c, tsl],
                                            osbs[0][0:64, :], rbss[0][:],
                                            mult)
                    ott = sm.tile([64, 512], BF, tag="ott", bufs=3)
                    nc.vector.tensor_tensor(ott[:], osbs[1][0:64, :],
                                            rbss[1][:], mult)
                    nc.sync.dma_start(OT[64:128, c, tsl], ott[:])
                    while fillers:
                        fillers.pop(0)()
                    return
                # stage-pipelined across the two heads: both psum->sbuf
                # copies issue back-to-back first, releasing the pO psB
                # slots before the next unit's accumulators need them (the
                # old per-head chain parked copy1 behind head0's gpsimd
                # wait in the DVE FIFO, stalling the next unit ~2us)
                osbs = []
                for hp, pO in ((0, pO0), (1, pO1)):
                    osb = sm.tile([65, 512], F32, tag="osb", bufs=3,
                                  name=f"osb_{hp}")
                    nc.vector.tensor_copy(osb[:], pO[:])
                    osbs.append(osb)
                recs = []
                for hp in (0, 1):
                    rec0 = sm.tile([1, 512], F32, tag="rec0", bufs=3,
                                   name=f"rec0_{hp}")
                    nc.sync.dma_start(rec0[:], osbs[hp][64:65, :])
                    recs.append(rec0)
                rins = []
                for hp in (0, 1):
                    rin = sm.tile([1, 512], F32, tag="rin", bufs=3,
                                  name=f"rin_{hp}")
                    nc.vector.reciprocal_approx_fast(rin[:], recs[hp][:])
                    rins.append(rin)
                rbss = []
                for hp in (0, 1):
                    rbs = sm.tile([64, 512], F32, tag="rbs", bufs=3,
                                  name=f"rbs_{hp}")
                    nc.gpsimd.partition_broadcast(rbs[:], rins[hp][:])
                    rbss.append(rbs)
                # head 0's partitions line up: write OT rows 0-63 directly
                nc.vector.tensor_tensor(OT[0:64, c, tsl],
                                        osbs[0][0:64, :], rbss[0][:], mult)
                # head 1 rows 64-127 need a cross-partition move (DMA)
                ott = sm.tile([64, 512], BF, tag="ott", bufs=3)
                nc.vector.tensor_tensor(ott[:], osbs[1][0:64, :],
                                        rbss[1][:], mult)
                nc.sync.dma_start(OT[64:128, c, tsl], ott[:])
                while fillers:
                    fillers.pop(0)()

            # ---- prologue: exactly what attention jc0 of unit (0,0)
            # needs, DMAs emitted in dependency order.  Only the c=0 halves
            # of the (0,0) span pairs run here (first-logits critical path);
            # the c=1 halves ride as early U00 fillers ----
            # DVE-paced warm-up: the HAM clock gate is cold (1.2 GHz) at
            # start and re-throttles after ~3.4us idle.  A ping-pong DVE
            # copy chain paces junk MMs across the initial DMA wait so the
            # PE is warm when the first real MMs fire.
            sa = xpool.tile([P, 512], BF, tag="scra", name="scr_a")
            sb = xpool.tile([P, 512], BF, tag="scrb", name="scr_b")
            nc.vector.memset(sa[:], 0.0)
            wps = psB.tile([P, 512], F32, tag="acc", name="warmps")
            for i in range(16):
                s_src, s_dst = (sa, sb) if i % 2 == 0 else (sb, sa)
                nc.vector.tensor_copy(s_dst[:], s_src[:])
                nc.tensor.matmul(wps[:], lhsT=s_dst[:, 0:128], rhs=s_dst[:],
                                 start=True, stop=True)

            # critical loads, Q first (its compute chain ends at the first
            # logits); biases deferred past the x data they don't gate
            nc.sync.dma_start(wq_t[:], d_wq[:])
            q00 = qk_pair(wq_t, bq_t, QT, d_xq, 0, 0)
            q00[0]()
            nc.sync.dma_start(wk_t[:], d_wk[:])
            k00 = qk_pair(wk_t, bk_t, KT, d_xk, 0, 0)
            k00[0]()
            nc.sync.dma_start(bq_t[:], d_bq[:])
            nc.sync.dma_start(bk_t[:], d_bk[:])
            nc.sync.dma_start(wv_t[:], d_wv[:])
            nc.sync.dma_start(bv_t[:], d_bv[:])
            v0_pre, v0 = v_group(0)
            v0_pre()
            # prefetches for U00's first consumers go out now, behind the
            # critical loads in the DMA FIFO
            vg = [None] + [v_group(sg) for sg in range(1, 8)]
            k01 = qk_pair(wk_t, bk_t, KT, d_xk, 0, 1)
            vg[1][0]()
            k01[0]()
            for s in q00[1] + k00[1] + v0:
                s()

            # ---- unit (0,0): carries the c=1 halves of the (0,0) pairs
            # (cheap, their x is resident), V chunks 1-7 (deadline jc=2sg),
            # and the c=0 K spans (deadline jc=4s) + Q span (0,1) c=0
            # (deadline: unit (0,1) start).  The far-deadline c=1 K/Q spans
            # are NOT here: they run in the ACT-paced later units, off
            # re-loaded x (those units have both PE and DMA slack) ----
            k10 = qk_pair(wk_t, bk_t, KT, d_xk, 1, 0)
            k11 = qk_pair(wk_t, bk_t, KT, d_xk, 1, 1)
            q01 = qk_pair(wq_t, bq_t, QT, d_xq, 0, 1)
            vg[2][0](); k10[0](); vg[3][0](); vg[4][0]()
            k11[0](); vg[5][0](); q01[0](); vg[6][0](); vg[7][0]()
            nc.sync.dma_start(wo_t[:], d_wo[:])
            nc.sync.dma_start(bo_t[:], d_bo[:])
            # explicit slots: V chunk sg must land by jc=2sg, K span s
            # (c=0, incl. the step-3 copyback) by jc=4s; the resident-x c=1
            # halves and Q (0,1) c=0 fill the remaining capacity
            V = [vg[s][1] if s else None for s in range(8)]
            slots00 = [
                [k00[2][0], k00[2][1]],
                [k00[2][2], k00[2][3], V[1][0]],
                [V[1][1], k01[1][0], k01[1][1]],
                [k01[1][2], k01[1][3], V[2][0]],
                [V[2][1], k10[1][0]],
                [k10[1][1], k10[1][2], V[3][0]],
                [V[3][1], k10[1][3]],
                [V[4][0], V[4][1]],
                [k11[1][0], k11[1][1]],
                [V[5][0], V[5][1]],
                [k11[1][2], k11[1][3]],
                [V[6][0], V[6][1]],
                [q01[1][0], q01[1][1]],
                [V[7][0], V[7][1], q01[1][2]],
                [q01[1][3], q00[2][0], q00[2][1]],
                [q00[2][2], q00[2][3]],
            ]
            attn_unit(0, 0, [], slots=slots00)

            # ---- unit (0,1): Q span (1,0), both slabs ----
            q10 = qk_pair(wq_t, bq_t, QT, d_xq, 1, 0)
            q10[0]()
            attn_unit(0, 1, q10[1] + q10[2], lead=3)

            # ---- remaining units: out-projection halves (<=12-16 fillers
            # per unit keeps the DVE off the per-jc critical path) plus the
            # c=1 K/Q spans off re-loaded x, placed just before their
            # consumers ----
            q11 = qk_pair(wq_t, bq_t, QT, d_xq, 1, 1)
            q11[0]()
            attn_unit(0, 2, q11[1] + q11[2] + out0_fill(0), lead=3)

            k01r = qk_pair(wk_t, bk_t, KT, d_xk, 0, 1)
            k01r[0]()
            attn_unit(0, 3, k01r[2] + q01[2] + out0_fill(1), lead=3)

            k10r = qk_pair(wk_t, bk_t, KT, d_xk, 1, 0)
            k11r = qk_pair(wk_t, bk_t, KT, d_xk, 1, 1)
            k10r[0](); k11r[0]()
            attn_unit(1, 0, k10r[2] + k11r[2] + out0_fill(2), lead=3)

            o10 = out1_fill(0)
            attn_unit(1, 1, out0_fill(3) + o10[:4], lead=3)
            attn_unit(1, 2, o10[4:] + out1_fill(1), lead=3)
            attn_unit(1, 3, out1_fill(2), lead=3, fast=True)
            for f in out1_fill(3):
                f()

    nc.compile()
    return nc


def _get_program():
    global _NC
    if _NC is None:
        _NC = _build_program()
    return _NC


def kernel(q, k, v, Wq, bq, Wk, bk, Wv, bv, Wo, bo):
    from concourse.bass_utils import run_bass_kernel_spmd

    q = np.asarray(q, np.float32)
    k = np.asarray(k, np.float32)
    v = np.asarray(v, np.float32)
    Wq = np.asarray(Wq, np.float32)
    Wk = np.asarray(Wk, np.float32)
    Wv = np.asarray(Wv, np.float32)
    Wo = np.asarray(Wo, np.float32)
    bq = np.asarray(bq, np.float32)
    bk = np.asarray(bk, np.float32)
    bv = np.asarray(bv, np.float32)
    bo = np.asarray(bo, np.float32)

    nc = _get_program()

    def tile_qk(xb):
        # [S, E] -> x^T tiled [4, 128, KC, 512]
        return np.ascontiguousarray(
            xb.T.reshape(KC, P, 4, 512).transpose(2, 1, 0, 3)).astype(BF16)

    def tile_v(xb):
        # [S, E] -> x^T tiled [8, 128, KC, 256]
        return np.ascontiguousarray(
            xb.T.reshape(KC, P, 8, GE).transpose(2, 1, 0, 3)).astype(BF16)

    xT = {"xqT": [tile_qk(q[b]) for b in range(2)],
          "xkT": [tile_qk(k[b]) for b in range(2)],
          "xvT": [tile_v(v[b]) for b in range(2)]}

    def wprep(W, scale=1.0):
        # [E, GE] slice -> [P, KC, GE] partition-major
        return [
            np.ascontiguousarray(
                (W[:, g * GE:(g + 1) * GE] * scale)
                .reshape(KC, P, GE).transpose(1, 0, 2)
            ).astype(BF16)
            for g in range(4)
        ]

    wq_g = wprep(Wq, 0.125)
    wk_g = wprep(Wk)
    wv_g = wprep(Wv)
    wo_g = [
        np.ascontiguousarray(
            Wo[g * GE:(g + 1) * GE, :].reshape(2, P, E).transpose(1, 0, 2)
        ).astype(BF16)
        for g in range(4)
    ]
    bq_g = [np.ascontiguousarray((bq[g * GE:(g + 1) * GE] * 0.125)
                                 .reshape(2, P).T).astype(np.float32)
            for g in range(4)]
    bk_g = [np.ascontiguousarray(bk[g * GE:(g + 1) * GE].reshape(2, P).T)
            .astype(np.float32) for g in range(4)]
    bv_g = [np.ascontiguousarray(np.broadcast_to(
        bv[g * GE:(g + 1) * GE].astype(np.float32), (P, GE))) for g in range(4)]
    bo_full = np.ascontiguousarray(
        np.broadcast_to(bo.astype(np.float32), (P, E)))
    bo_zero = np.zeros((P, E), np.float32)

    in_maps = []
    for c in range(NCORES):
        b, g = divmod(c, 4)
        in_maps.append({
            "xqT": xT["xqT"][b],
            "xkT": xT["xkT"][b],
            "xvT": xT["xvT"][b],
            "wq": wq_g[g], "wk": wk_g[g], "wv": wv_g[g], "wo": wo_g[g],
            "bqs": bq_g[g], "bks": bk_g[g], "bvb": bv_g[g],
            "bob": bo_full if g == 0 else bo_zero,
        })

    res = run_bass_kernel_spmd(nc, in_maps, list(range(NCORES)),
                               **_RUN_KWARGS)
    globals()["LAST_RESULTS"] = res

    parts = [res.results[c]["y"] for c in range(NCORES)]
    out = np.stack([
        parts[0] + parts[1] + parts[2] + parts[3],
        parts[4] + parts[5] + parts[6] + parts[7],
    ]).astype(np.float32)
    return out


# test-harness hooks (kernel.py itself never enables tracing)
_RUN_KWARGS = {}
LAST_RESULTS = None
